# revision 5
# baseline (speedup 1.0000x reference)
"""Trainium2 Bass kernel: per-batch global average pooling (segment mean).

reference: sums = segment_sum(features, batch_index, 32); out = sums / counts

Strategy (8 NeuronCores, SPMD):
  - Shard the 4M rows across 8 cores. Shards overlap slightly so every
    shard is exactly P*sum(TPCS) rows (no host-side padding copy of the
    1 GB features array — shards are numpy views). Overlapped rows are
    "disowned" on all but one core by setting their batch index to the
    sentinel 32 in the per-core index image (host-built, 8 MB total).
  - Per core, per 8192-row chunk (2 MB of HBM): SWDGE cast-DMA the fp32
    features into SBUF as bf16 [128 partitions, 64 rows x 64]. The cast
    halves SBUF traffic and makes the matmuls bf16: fp32 matmuls run in
    LOW_HIGH mode (2x LDWEIGHTS) and were measured to slow the
    concurrent DMA stream from ~422 to ~320 GB/s. 2 MB chunks amortize
    the SWDGE 4-semaphore lane cycle (~7 us completion-receipt + wake +
    descriptor-gen per recycle), which capped 1 MB chunks at ~375 GB/s.
  - VectorE builds onehot[p, t*32+s] = (idx==s) in bf16 with one
    is_equal against a host-provided iota image (loading iota as an
    input keeps the in-order gpsimd queue free to start streaming
    immediately). TensorE runs one bf16 matmul per 128-row tile:
    onehot_t.T @ feat_t accumulating into PSUM fp32, rotating over four
    32-partition PSUM bands (tile_position column packing).
  - Counts and the cross-band/cross-core reduction happen on the host:
    counts = bincount(batch_index) exactly; the kernel outputs the raw
    4 PSUM bands as [128, 64] and the host folds [4, 32, 64] -> [32, 64].
  - bf16 feature rounding is unbiased and averages out over ~125k rows
    per segment: measured end-to-end relative error ~1.6e-3 (budget 2e-2).
"""

import sys

for _p in ("/opt/trn_rl_repo",):
    if _p not in sys.path:
        sys.path.insert(0, _p)

import numpy as np

import concourse.bass as bass
import concourse.tile as tile
from concourse import bacc
from concourse import mybir
from concourse.bass_utils import run_bass_kernel_spmd

P = 128          # SBUF partitions
D = 64           # feature dim
S = 32           # number of segments
SENTINEL = float(S)  # batch index value that matches no segment
NBANDS = 4       # PSUM bands / PE column groups used for matmul packing

N_CORES = 8
N_ROWS = 4_000_000
TPC = 128                    # rows per partition per full chunk (= tiles per chunk)
TPCS = [TPC] * 30 + [67]     # 30*128+67 = 3907 tiles -> shard 500096 rows
SHARD = P * sum(TPCS)        # 500096 rows per core (8*SHARD = 4000768; ~0.02% overlap)

FEAT_BUFS = 9
OH_BUFS = 4


def build_nc(tpcs=None) -> bass.Bass:
    if tpcs is None:
        tpcs = TPCS
    tmax = max(tpcs)
    w = sum(tpcs)
    nc = bacc.Bacc(None)
    feat = nc.declare_dram_parameter(
        "feat", [P * w, D], mybir.dt.float32, isOutput=False
    )
    idx = nc.declare_dram_parameter("idx", [P, w], mybir.dt.bfloat16, isOutput=False)
    iota = nc.declare_dram_parameter(
        "iota", [P, tmax * S], mybir.dt.bfloat16, isOutput=False
    )
    out = nc.declare_dram_parameter("out", [P, D], mybir.dt.float32, isOutput=True)

    # last (chunk, tile) per PSUM band, for the stop flags
    last_of_band = {}
    for c, tpc in enumerate(tpcs):
        for t in range(tpc):
            last_of_band[t % NBANDS] = (c, t)

    with tile.TileContext(nc) as tc:
        with (
            tc.tile_pool(name="const", bufs=1) as cpool,
            tc.tile_pool(name="feat", bufs=1) as fpool,
            tc.tile_pool(name="oh", bufs=1) as opool,
            tc.tile_pool(name="psum", bufs=1, space="PSUM") as ppool,
        ):
            # index image + iota ride the scalar (ACT) HWDGE ring so the
            # gpsimd SWDGE ring starts streaming features immediately
            idx_sb = cpool.tile([P, w], mybir.dt.bfloat16)
            nc.scalar.dma_start(out=idx_sb[:], in_=idx[:])
            iota_f = cpool.tile([P, tmax * S], mybir.dt.bfloat16)
            nc.scalar.dma_start(out=iota_f[:], in_=iota[:])

            ftiles = [
                fpool.tile([P, tmax * D], mybir.dt.bfloat16, tag=f"f{j}", name=f"ft{j}")
                for j in range(FEAT_BUFS)
            ]
            ohtiles = [
                opool.tile([P, tmax * S], mybir.dt.bfloat16, tag=f"o{j}", name=f"oh{j}")
                for j in range(OH_BUFS)
            ]

            # one PSUM tile per band so the 4 interleaved accumulation
            # groups live in distinct zero-regions
            psum_bands = [
                ppool.tile([P, D], mybir.dt.float32, name=f"psband{b}")
                for b in range(NBANDS)
            ]

            row = 0   # feature-row base (in per-partition units)
            col = 0   # idx-image column base
            for c, tpc in enumerate(tpcs):
                chunk = P * tpc
                ft = ftiles[c % FEAT_BUFS]
                oh = ohtiles[c % OH_BUFS]
                src = feat[row : row + chunk, :].rearrange(
                    "(pp t) dd -> pp (t dd)", pp=P
                )
                # SWDGE cast-DMA: fp32 in HBM -> bf16 in SBUF
                nc.gpsimd.dma_start(out=ft[:, : tpc * D], in_=src)
                nc.vector.tensor_tensor(
                    out=oh[:, : tpc * S].rearrange("p (t s) -> p t s", s=S),
                    in0=iota_f[:, : tpc * S].rearrange("p (t s) -> p t s", s=S),
                    in1=idx_sb[:, col : col + tpc].to_broadcast([P, tpc, S]),
                    op=mybir.AluOpType.is_equal,
                )
                for t in range(tpc):
                    b = t % NBANDS
                    nc.tensor.matmul(
                        out=psum_bands[b][b * S : (b + 1) * S, :],
                        lhsT=oh[:, t * S : (t + 1) * S],
                        rhs=ft[:, t * D : (t + 1) * D],
                        start=(c == 0 and t < NBANDS),
                        stop=(last_of_band[b] == (c, t)),
                        tile_position=(0, b * S),
                    )
                row += chunk
                col += tpc

            # tail: copy the 4 PSUM bands into one [128, 64] SBUF tile and
            # store raw; the host folds the bands and divides by counts
            out_sb = cpool.tile([P, D], mybir.dt.float32)
            for b in range(NBANDS):
                nc.vector.tensor_copy(
                    out_sb[b * S : (b + 1) * S, :],
                    psum_bands[b][b * S : (b + 1) * S, :],
                )
            nc.sync.dma_start(out=out[:], in_=out_sb[:])

    nc.compile()
    return nc


def shard_plan(n_rows: int = N_ROWS, shard: int = SHARD, n_cores: int = N_CORES):
    """Overlapping shard starts + per-core disowned-head lengths."""
    base = n_rows - shard
    starts = [i * base // (n_cores - 1) for i in range(n_cores)]
    disown = [0] * n_cores
    for i in range(1, n_cores):
        disown[i] = (starts[i - 1] + shard) - starts[i]
        assert 0 <= disown[i] <= shard
    assert starts[-1] + shard == n_rows
    return starts, disown


def build_idx_image(batch_index: np.ndarray, start: int, disown: int,
                    tpcs=None) -> np.ndarray:
    import ml_dtypes

    if tpcs is None:
        tpcs = TPCS
    shard = P * sum(tpcs)
    sidx = batch_index[start : start + shard].astype(np.float32)  # exact for 0..32
    if disown:
        sidx[:disown] = SENTINEL
    img = np.empty((P, sum(tpcs)), dtype=np.float32)
    row, col = 0, 0
    for tpc in tpcs:
        img[:, col : col + tpc] = sidx[row : row + P * tpc].reshape(P, tpc)
        row += P * tpc
        col += tpc
    return np.ascontiguousarray(img.astype(ml_dtypes.bfloat16))


def build_iota(tmax: int = TPC) -> np.ndarray:
    import ml_dtypes

    row = np.tile(np.arange(S, dtype=np.float32), tmax)  # [tmax*S]: t*S+s -> s
    img = np.broadcast_to(row, (P, tmax * S))
    return np.ascontiguousarray(img.astype(ml_dtypes.bfloat16))


_NC_CACHE: dict = {}


def _get_nc():
    if "nc" not in _NC_CACHE:
        _NC_CACHE["nc"] = build_nc()
    return _NC_CACHE["nc"]


def kernel(features: np.ndarray, batch_index: np.ndarray, **run_kwargs) -> np.ndarray:
    assert features.shape == (N_ROWS, D), features.shape
    assert batch_index.shape == (N_ROWS,), batch_index.shape
    features = np.asarray(features, dtype=np.float32)
    batch_index = np.asarray(batch_index)

    starts, disown = shard_plan()
    iota = build_iota()
    in_maps = []
    for i in range(N_CORES):
        in_maps.append(
            {
                "feat": features[starts[i] : starts[i] + SHARD],
                "idx": build_idx_image(batch_index, starts[i], disown[i]),
                "iota": iota,
            }
        )

    nc = _get_nc()
    res = run_bass_kernel_spmd(nc, in_maps, list(range(N_CORES)), **run_kwargs)
    total = np.zeros((S, D), dtype=np.float64)
    for r in res.results:
        total += r["out"].astype(np.float64).reshape(NBANDS, S, D).sum(axis=0)
    counts = np.bincount(np.asarray(batch_index).astype(np.int64), minlength=S)
    out = total / counts[:, None]
    kernel.last_results = res  # expose exec_time/trace to the caller
    return out.astype(np.float32)


# revision 6
# speedup vs baseline: 1.0830x; 1.0830x over previous
"""Trainium2 Bass kernel: per-batch global average pooling (segment mean).

reference: sums = segment_sum(features, batch_index, 32); out = sums / counts

Strategy (8 NeuronCores, SPMD):
  - Shard the 4M rows across 8 cores. Shards overlap slightly so every
    shard is exactly P*sum(TPCS) rows (no host-side padding copy of the
    1 GB features array — shards are numpy views). Overlapped rows are
    "disowned" on all but one core by setting their batch index to the
    sentinel 32 in the per-core index image (host-built, 8 MB total).
  - Per core, per 8192-row chunk (2 MB of HBM): SWDGE cast-DMA the fp32
    features into SBUF as bf16 [128 partitions, 64 rows x 64]. The cast
    halves SBUF traffic and makes the matmuls bf16: fp32 matmuls run in
    LOW_HIGH mode (2x LDWEIGHTS) and were measured to slow the
    concurrent DMA stream from ~422 to ~320 GB/s. 2 MB chunks amortize
    the SWDGE 4-semaphore lane cycle (~7 us completion-receipt + wake +
    descriptor-gen per recycle), which capped 1 MB chunks at ~375 GB/s.
  - VectorE builds onehot[p, t*32+s] = (idx==s) in bf16 with one
    is_equal against a host-provided iota image (loading iota as an
    input keeps the in-order gpsimd queue free to start streaming
    immediately). TensorE runs one bf16 matmul per 128-row tile:
    onehot_t.T @ feat_t accumulating into PSUM fp32, rotating over four
    32-partition PSUM bands (tile_position column packing).
  - Counts and the cross-band/cross-core reduction happen on the host:
    counts = bincount(batch_index) exactly; the kernel outputs the raw
    4 PSUM bands as [128, 64] and the host folds [4, 32, 64] -> [32, 64].
  - bf16 feature rounding is unbiased and averages out over ~125k rows
    per segment: measured end-to-end relative error ~1.6e-3 (budget 2e-2).
"""

import sys

for _p in ("/opt/trn_rl_repo",):
    if _p not in sys.path:
        sys.path.insert(0, _p)

import numpy as np

import concourse.bass as bass
import concourse.tile as tile
from concourse import bacc
from concourse import mybir
from concourse.bass_utils import run_bass_kernel_spmd

P = 128          # SBUF partitions
D = 64           # feature dim
S = 32           # number of segments
SENTINEL = float(S)  # batch index value that matches no segment
NBANDS = 4       # PSUM bands / PE column groups used for matmul packing

N_CORES = 8
N_ROWS = 4_000_000
TPC = 96                     # rows per partition per full chunk (= tiles per chunk)
TPCS = [TPC] * 40 + [67]     # 40*96+67 = 3907 tiles -> shard 500096 rows
SHARD = P * sum(TPCS)        # 500096 rows per core (8*SHARD = 4000768; ~0.02% overlap)

FEAT_BUFS = 12
OH_BUFS = 5


def build_nc(tpcs=None) -> bass.Bass:
    if tpcs is None:
        tpcs = TPCS
    tmax = max(tpcs)
    w = sum(tpcs)
    nc = bacc.Bacc(None)
    feat = nc.declare_dram_parameter(
        "feat", [P * w, D], mybir.dt.float32, isOutput=False
    )
    idx = nc.declare_dram_parameter("idx", [P, w], mybir.dt.bfloat16, isOutput=False)
    iota = nc.declare_dram_parameter(
        "iota", [P, tmax * S], mybir.dt.bfloat16, isOutput=False
    )
    out = nc.declare_dram_parameter("out", [P, D], mybir.dt.float32, isOutput=True)

    # last (chunk, tile) per PSUM band, for the stop flags
    last_of_band = {}
    for c, tpc in enumerate(tpcs):
        for t in range(tpc):
            last_of_band[t % NBANDS] = (c, t)

    with tile.TileContext(nc) as tc:
        with (
            tc.tile_pool(name="const", bufs=1) as cpool,
            tc.tile_pool(name="feat", bufs=1) as fpool,
            tc.tile_pool(name="oh", bufs=1) as opool,
            tc.tile_pool(name="psum", bufs=1, space="PSUM") as ppool,
        ):
            # index image + iota ride the scalar (ACT) HWDGE ring so the
            # gpsimd SWDGE ring starts streaming features immediately
            idx_sb = cpool.tile([P, w], mybir.dt.bfloat16)
            nc.scalar.dma_start(out=idx_sb[:], in_=idx[:])
            iota_f = cpool.tile([P, tmax * S], mybir.dt.bfloat16)
            nc.scalar.dma_start(out=iota_f[:], in_=iota[:])

            ftiles = [
                fpool.tile([P, tmax * D], mybir.dt.bfloat16, tag=f"f{j}", name=f"ft{j}")
                for j in range(FEAT_BUFS)
            ]
            ohtiles = [
                opool.tile([P, tmax * S], mybir.dt.bfloat16, tag=f"o{j}", name=f"oh{j}")
                for j in range(OH_BUFS)
            ]

            # one PSUM tile per band so the 4 interleaved accumulation
            # groups live in distinct zero-regions
            psum_bands = [
                ppool.tile([P, D], mybir.dt.float32, name=f"psband{b}")
                for b in range(NBANDS)
            ]

            row = 0   # feature-row base (in per-partition units)
            col = 0   # idx-image column base
            for c, tpc in enumerate(tpcs):
                chunk = P * tpc
                ft = ftiles[c % FEAT_BUFS]
                oh = ohtiles[c % OH_BUFS]
                src = feat[row : row + chunk, :].rearrange(
                    "(pp t) dd -> pp (t dd)", pp=P
                )
                # SWDGE cast-DMA: fp32 in HBM -> bf16 in SBUF
                nc.gpsimd.dma_start(out=ft[:, : tpc * D], in_=src)
                nc.vector.tensor_tensor(
                    out=oh[:, : tpc * S].rearrange("p (t s) -> p t s", s=S),
                    in0=iota_f[:, : tpc * S].rearrange("p (t s) -> p t s", s=S),
                    in1=idx_sb[:, col : col + tpc].to_broadcast([P, tpc, S]),
                    op=mybir.AluOpType.is_equal,
                )
                for t in range(tpc):
                    b = t % NBANDS
                    nc.tensor.matmul(
                        out=psum_bands[b][b * S : (b + 1) * S, :],
                        lhsT=oh[:, t * S : (t + 1) * S],
                        rhs=ft[:, t * D : (t + 1) * D],
                        start=(c == 0 and t < NBANDS),
                        stop=(last_of_band[b] == (c, t)),
                        tile_position=(0, b * S),
                    )
                row += chunk
                col += tpc

            # tail: copy the 4 PSUM bands into one [128, 64] SBUF tile and
            # store raw; the host folds the bands and divides by counts
            out_sb = cpool.tile([P, D], mybir.dt.float32)
            for b in range(NBANDS):
                nc.vector.tensor_copy(
                    out_sb[b * S : (b + 1) * S, :],
                    psum_bands[b][b * S : (b + 1) * S, :],
                )
            nc.sync.dma_start(out=out[:], in_=out_sb[:])

    nc.compile()
    return nc


def shard_plan(n_rows: int = N_ROWS, shard: int = SHARD, n_cores: int = N_CORES):
    """Overlapping shard starts + per-core disowned-head lengths."""
    base = n_rows - shard
    starts = [i * base // (n_cores - 1) for i in range(n_cores)]
    disown = [0] * n_cores
    for i in range(1, n_cores):
        disown[i] = (starts[i - 1] + shard) - starts[i]
        assert 0 <= disown[i] <= shard
    assert starts[-1] + shard == n_rows
    return starts, disown


def build_idx_image(batch_index: np.ndarray, start: int, disown: int,
                    tpcs=None) -> np.ndarray:
    import ml_dtypes

    if tpcs is None:
        tpcs = TPCS
    shard = P * sum(tpcs)
    sidx = batch_index[start : start + shard].astype(np.float32)  # exact for 0..32
    if disown:
        sidx[:disown] = SENTINEL
    img = np.empty((P, sum(tpcs)), dtype=np.float32)
    row, col = 0, 0
    for tpc in tpcs:
        img[:, col : col + tpc] = sidx[row : row + P * tpc].reshape(P, tpc)
        row += P * tpc
        col += tpc
    return np.ascontiguousarray(img.astype(ml_dtypes.bfloat16))


def build_iota(tmax: int = TPC) -> np.ndarray:
    import ml_dtypes

    row = np.tile(np.arange(S, dtype=np.float32), tmax)  # [tmax*S]: t*S+s -> s
    img = np.broadcast_to(row, (P, tmax * S))
    return np.ascontiguousarray(img.astype(ml_dtypes.bfloat16))


_NC_CACHE: dict = {}


def _get_nc():
    if "nc" not in _NC_CACHE:
        _NC_CACHE["nc"] = build_nc()
    return _NC_CACHE["nc"]


def kernel(features: np.ndarray, batch_index: np.ndarray, **run_kwargs) -> np.ndarray:
    assert features.shape == (N_ROWS, D), features.shape
    assert batch_index.shape == (N_ROWS,), batch_index.shape
    features = np.asarray(features, dtype=np.float32)
    batch_index = np.asarray(batch_index)

    starts, disown = shard_plan()
    iota = build_iota()
    in_maps = []
    for i in range(N_CORES):
        in_maps.append(
            {
                "feat": features[starts[i] : starts[i] + SHARD],
                "idx": build_idx_image(batch_index, starts[i], disown[i]),
                "iota": iota,
            }
        )

    nc = _get_nc()
    res = run_bass_kernel_spmd(nc, in_maps, list(range(N_CORES)), **run_kwargs)
    total = np.zeros((S, D), dtype=np.float64)
    for r in res.results:
        total += r["out"].astype(np.float64).reshape(NBANDS, S, D).sum(axis=0)
    counts = np.bincount(np.asarray(batch_index).astype(np.int64), minlength=S)
    out = total / counts[:, None]
    kernel.last_results = res  # expose exec_time/trace to the caller
    return out.astype(np.float32)
